# revision 3
# baseline (speedup 1.0000x reference)
"""Sparse single-head attention (QKV proj + key-padding mask + softmax) on 8 trn2 cores.

Math per batch element b (one NeuronCore each):
    qh = q @ Wq + bq ; kh = k @ Wk + bk ; vh = v @ Wv + bv        [S, 64]
    scores = qh @ kh^T / 8 ; scores[:, mask==0] = -1e10
    out = softmax(scores, -1) @ vh                                 [S, 64]

Device strategy:
  - Host gathers the unmasked k/v rows (mask is ~50% zeros) and pads to SK
    (multiple of 512); padded keys get an additive bias of -1e10 so their
    exp() underflows to exactly 0 - identical math to the reference.
  - All of q/k/v must be d-major on chip (PE contracts over partitions), so
    natural [128, 512] tiles are PE-transposed in 128x128 blocks.
  - qh^T is augmented with a row of ones and kh^T with a row of mask biases:
    the scores matmul then fuses the additive mask for free.  1/sqrt(64) is
    folded into Wq/bq on the host.
  - scores are computed TRANSPOSED ([k, q] layout, k on partitions): softmax
    exp is layout-agnostic, the sum over k comes for free from a ones-column
    appended to vh (row 64 of the output accumulator = sum of exps), and
    attn^T is exactly what the out-matmul needs as lhsT - no per-tile
    attention transposes.
  - exp() is not max-stabilized: scores ~ N(0, 0.11) for this problem's
    input distribution, far inside fp32 exp range; masked lanes are -1e10
    which underflows to +0 exactly like the stabilized reference.
  - Final [64, q] -> [q, 64] transpose + row-scale by 1/sum on 4 small tiles
    per q-block.
"""

import numpy as np

import concourse.bass as bass
import concourse.tile as tile
from concourse import bacc, mybir
from concourse.bass_utils import run_bass_kernel_spmd
from concourse.masks import make_identity

F32 = mybir.dt.float32
S = 4096  # query rows per core
D = 512  # model dim
DK = 64  # head dim (q/k and v)
N_CORES = 8


def _build_nc(SK: int):
    """Build the single-core Bass program (same program on all 8 cores)."""
    assert SK % 512 == 0
    SKC = SK // 128  # 128-row key chunks
    NPAIR = SKC // 2

    nc = bacc.Bacc("TRN2", target_bir_lowering=False, debug=False)

    q_d = nc.dram_tensor("q", [S, D], F32, kind="ExternalInput").ap()
    kg_d = nc.dram_tensor("kg", [SK, D], F32, kind="ExternalInput").ap()
    vg_d = nc.dram_tensor("vg", [SK, D], F32, kind="ExternalInput").ap()
    mb_d = nc.dram_tensor("mbias", [1, SK], F32, kind="ExternalInput").ap()
    wq_d = nc.dram_tensor("wq", [D, DK], F32, kind="ExternalInput").ap()
    wk_d = nc.dram_tensor("wk", [D, DK], F32, kind="ExternalInput").ap()
    wv_d = nc.dram_tensor("wv", [D, DK + 1], F32, kind="ExternalInput").ap()
    bq_d = nc.dram_tensor("bq", [DK, 1], F32, kind="ExternalInput").ap()
    bk_d = nc.dram_tensor("bk", [DK, 1], F32, kind="ExternalInput").ap()
    bv_d = nc.dram_tensor("bv", [1, DK + 1], F32, kind="ExternalInput").ap()
    out_d = nc.dram_tensor("out", [S, DK], F32, kind="ExternalOutput").ap()

    with tile.TileContext(nc) as tc:
        with (
            tc.tile_pool(name="persist", bufs=1) as persist,
            tc.tile_pool(name="consts", bufs=1) as consts,
        ):
            ident = consts.tile([128, 128], F32)
            make_identity(nc, ident[:, :])
            onesrow = consts.tile([1, 128], F32)
            nc.vector.memset(onesrow[:, :], 1.0)

            wq = consts.tile([128, 4, DK], F32)
            wk = consts.tile([128, 4, DK], F32)
            wv = consts.tile([128, 4, DK + 1], F32)
            nc.sync.dma_start(wq[:, :, :], wq_d.rearrange("(c p) k -> p c k", p=128))
            nc.sync.dma_start(wk[:, :, :], wk_d.rearrange("(c p) k -> p c k", p=128))
            nc.sync.dma_start(wv[:, :, :], wv_d.rearrange("(c p) k -> p c k", p=128))
            bq = consts.tile([DK, 1], F32)
            bk = consts.tile([DK, 1], F32)
            bv = consts.tile([1, DK + 1], F32)
            nc.sync.dma_start(bq[:, :], bq_d)
            nc.sync.dma_start(bk[:, :], bk_d)
            nc.sync.dma_start(bv[:, :], bv_d)

            qhT = persist.tile([DK + 1, S], F32)  # row 64 = ones
            khT = persist.tile([DK + 1, SK], F32)  # row 64 = mask bias
            vh = persist.tile([128, SKC, DK + 1], F32)  # col 64 = ones
            nc.vector.memset(qhT[DK : DK + 1, :], 1.0)
            nc.sync.dma_start(khT[DK : DK + 1, :], mb_d)

            # ---- Phase 1: projections (k, v, then q), software-pipelined ----
            with (
                tc.tile_pool(name="nat", bufs=2) as nat_pool,
                tc.tile_pool(name="xt", bufs=2) as xt_pool,
                tc.tile_pool(name="ps_tr", bufs=4, space="PSUM") as ps_tr,
                tc.tile_pool(name="ps_pr", bufs=2, space="PSUM") as ps_pr,
            ):
                evac_ctr = [0]

                def load_and_transpose(src_ap, sb):
                    """DMA a 512-row block and PE-transpose it to d-major."""
                    nat = nat_pool.tile([128, 4, D], F32, tag="nat")
                    nc.sync.dma_start(
                        nat[:, :, :],
                        src_ap[sb * 512 : (sb + 1) * 512, :].rearrange(
                            "(t p) d -> p t d", p=128
                        ),
                    )
                    xt = xt_pool.tile([128, 4, 512], F32, tag="xt")
                    for t in range(4):
                        for c in range(4):
                            ps = ps_tr.tile([128, 128], F32, tag="tr")
                            nc.tensor.transpose(
                                ps[:, :], nat[:, t, c * 128 : (c + 1) * 128], ident[:, :]
                            )
                            # round-robin evacuation 2:1 DVE:ACT
                            eng = nc.scalar if evac_ctr[0] % 3 == 2 else nc.vector
                            evac_ctr[0] += 1
                            if eng is nc.scalar:
                                nc.scalar.copy(xt[:, c, t * 128 : (t + 1) * 128], ps[:, :])
                            else:
                                nc.vector.tensor_copy(
                                    xt[:, c, t * 128 : (t + 1) * 128], ps[:, :]
                                )
                    return xt

                def project_qk(xt, dst, bias, sb):
                    """dst[0:64, sb*512:+512] = W^T @ x^T + bias (per-partition)."""
                    w = wq if dst is qhT else wk
                    ps = ps_pr.tile([DK, 512], F32, tag="pr")
                    for c in range(4):
                        nc.tensor.matmul(
                            ps[:, :],
                            w[:, c, :],
                            xt[:, c, :],
                            start=(c == 0),
                            stop=(c == 3),
                        )
                    nc.scalar.activation(
                        dst[0:DK, sb * 512 : (sb + 1) * 512],
                        ps[:, :],
                        mybir.ActivationFunctionType.Identity,
                        bias=bias[:, :],
                    )

                def project_v(xt, sb):
                    """vh[:, sb*4+t, :] = v_block^T^T @ Wv_aug + bv_aug (ones col)."""
                    for t in range(4):
                        ps = ps_pr.tile([128, DK + 1], F32, tag="prv")
                        for c in range(4):
                            nc.tensor.matmul(
                                ps[:, :],
                                xt[:, c, t * 128 : (t + 1) * 128],
                                wv[:, c, :],
                                start=(c == 0),
                                stop=False,
                            )
                        nc.tensor.matmul(
                            ps[:, :], onesrow[:, :], bv[:, :], start=False, stop=True
                        )
                        nc.vector.tensor_copy(vh[:, sb * 4 + t, :], ps[:, :])

                # interleave: transpose block i+1 is emitted before projecting
                # block i so PE never stalls on the evacuation copies.
                work = (
                    [("k", sb) for sb in range(SK // 512)]
                    + [("v", sb) for sb in range(SK // 512)]
                    + [("q", sb) for sb in range(S // 512)]
                )
                pending = None  # (kind, sb, xt)
                for kind, sb in work:
                    src = {"k": kg_d, "v": vg_d, "q": q_d}[kind]
                    xt = load_and_transpose(src, sb)
                    if pending is not None:
                        pk, psb, pxt = pending
                        if pk == "k":
                            project_qk(pxt, khT, bk, psb)
                        elif pk == "q":
                            project_qk(pxt, qhT, bq, psb)
                        else:
                            project_v(pxt, psb)
                    pending = (kind, sb, xt)
                pk, psb, pxt = pending
                project_qk(pxt, qhT, bq, psb)

            # ---- Phase 2: scores^T -> exp -> out^T accumulation ----
            with (
                tc.tile_pool(name="ps_s", bufs=2, space="PSUM") as ps_s,
                tc.tile_pool(name="ps_o", bufs=2, space="PSUM") as ps_o,
                tc.tile_pool(name="ps_f", bufs=2, space="PSUM") as ps_f,
                tc.tile_pool(name="expp", bufs=3) as exp_pool,
                tc.tile_pool(name="otp", bufs=2) as ot_pool,
                tc.tile_pool(name="recp", bufs=8) as rec_pool,
                tc.tile_pool(name="outp", bufs=2) as out_pool,
            ):
                for qb in range(S // 512):
                    qs = qhT[:, qb * 512 : (qb + 1) * 512]
                    po = ps_o.tile([DK + 1, 512], F32, tag="o")
                    prev = None  # (exp_tile, kp)
                    for kp in range(NPAIR):
                        pscore = ps_s.tile([128, 1024], F32, tag="s")
                        for h in range(2):
                            kc = 2 * kp + h
                            nc.tensor.matmul(
                                pscore[:, h * 512 : (h + 1) * 512],
                                khT[:, kc * 128 : (kc + 1) * 128],
                                qs,
                                start=True,
                                stop=True,
                            )
                        et = exp_pool.tile([128, 1024], F32, tag="e")
                        nc.scalar.activation(
                            et[:, :], pscore[:, :], mybir.ActivationFunctionType.Exp
                        )
                        if prev is not None:
                            pet, pkp = prev
                            for h in range(2):
                                kc = 2 * pkp + h
                                nc.tensor.matmul(
                                    po[:, :],
                                    vh[:, kc, :],
                                    pet[:, h * 512 : (h + 1) * 512],
                                    start=(kc == 0),
                                    stop=False,
                                )
                        prev = (et, kp)
                    pet, pkp = prev
                    for h in range(2):
                        kc = 2 * pkp + h
                        nc.tensor.matmul(
                            po[:, :],
                            vh[:, kc, :],
                            pet[:, h * 512 : (h + 1) * 512],
                            start=(kc == 0),
                            stop=(h == 1),
                        )

                    # finalize q-block: transpose back, scale rows by 1/sum
                    ot = ot_pool.tile([DK + 1, 512], F32, tag="ot")
                    nc.vector.tensor_copy(ot[:, :], po[:, :])
                    ostage = out_pool.tile([128, 4, DK], F32, tag="os")
                    for t in range(4):
                        pf = ps_f.tile([128, DK + 1], F32, tag="f")
                        nc.tensor.transpose(
                            pf[:, :],
                            ot[:, t * 128 : (t + 1) * 128],
                            ident[0 : DK + 1, 0 : DK + 1],
                        )
                        rec = rec_pool.tile([128, 1], F32, tag="r")
                        nc.vector.reciprocal(rec[:, :], pf[:, DK : DK + 1])
                        nc.vector.tensor_scalar_mul(
                            ostage[:, t, :], pf[:, 0:DK], rec[:, :]
                        )
                    nc.sync.dma_start(
                        out_d[qb * 512 : (qb + 1) * 512, :].rearrange(
                            "(t p) v -> p t v", p=128
                        ),
                        ostage[:, :, :],
                    )

    nc.compile()
    return nc


_NC_CACHE: dict = {}


def prepare(inputs):
    """Host-side preprocessing: returns (nc, in_maps)."""
    q = np.ascontiguousarray(inputs["q"], dtype=np.float32)
    k = np.ascontiguousarray(inputs["k"], dtype=np.float32)
    v = np.ascontiguousarray(inputs["v"], dtype=np.float32)
    mask = np.asarray(inputs["mask"])
    Wq = np.asarray(inputs["Wq"], dtype=np.float32)
    bq = np.asarray(inputs["bq"], dtype=np.float32)
    Wk = np.asarray(inputs["Wk"], dtype=np.float32)
    bk = np.asarray(inputs["bk"], dtype=np.float32)
    Wv = np.asarray(inputs["Wv"], dtype=np.float32)
    bv = np.asarray(inputs["bv"], dtype=np.float32)
    B = q.shape[0]
    assert q.shape == (B, S, D) and B == N_CORES

    # gather unmasked key/value rows per batch; pad to a common SK
    idxs = [np.nonzero(mask[b])[0] for b in range(B)]
    max_cnt = max(len(ix) for ix in idxs)
    SK = ((max_cnt + 511) // 512) * 512
    SK = max(SK, 512)

    scale = 1.0 / np.sqrt(np.float32(DK))
    Wq8 = (Wq * scale).astype(np.float32)
    bq8 = (bq * scale).astype(np.float32).reshape(DK, 1)
    bk2 = bk.astype(np.float32).reshape(DK, 1)
    Wv_aug = np.concatenate([Wv, np.zeros((D, 1), np.float32)], axis=1)
    bv_aug = np.concatenate([bv, np.ones(1, np.float32)]).reshape(1, DK + 1)

    in_maps = []
    for b in range(B):
        ix = idxs[b]
        cnt = len(ix)
        kg = np.zeros((SK, D), np.float32)
        vg = np.zeros((SK, D), np.float32)
        kg[:cnt] = k[b][ix]
        vg[:cnt] = v[b][ix]
        mb = np.zeros((1, SK), np.float32)
        mb[0, cnt:] = -1e10
        in_maps.append(
            dict(
                q=q[b],
                kg=kg,
                vg=vg,
                mbias=mb,
                wq=Wq8,
                wk=Wk.astype(np.float32),
                wv=Wv_aug,
                bq=bq8,
                bk=bk2,
                bv=bv_aug,
            )
        )

    if SK not in _NC_CACHE:
        _NC_CACHE[SK] = _build_nc(SK)
    return _NC_CACHE[SK], in_maps


def kernel(**inputs) -> np.ndarray:
    nc, in_maps = prepare(inputs)
    res = run_bass_kernel_spmd(nc, in_maps, list(range(N_CORES)))
    out = np.stack([res.results[b]["out"] for b in range(len(in_maps))], axis=0)
    return out.astype(np.float32)


if __name__ == "__main__":
    rng = np.random.default_rng(0)
    ins = {
        "q": rng.standard_normal((8, S, D), dtype=np.float32),
        "k": rng.standard_normal((8, S, D), dtype=np.float32),
        "v": rng.standard_normal((8, S, D), dtype=np.float32),
        "mask": rng.integers(0, 2, size=(8, S)).astype(np.int32),
        "Wq": rng.uniform(-0.04, 0.04, (D, DK)).astype(np.float32),
        "bq": rng.uniform(-0.04, 0.04, DK).astype(np.float32),
        "Wk": rng.uniform(-0.04, 0.04, (D, DK)).astype(np.float32),
        "bk": rng.uniform(-0.04, 0.04, DK).astype(np.float32),
        "Wv": rng.uniform(-0.04, 0.04, (D, DK)).astype(np.float32),
        "bv": rng.uniform(-0.04, 0.04, DK).astype(np.float32),
    }
    out = kernel(**ins)
    print("out", out.shape, out.dtype)
